# revision 36
# baseline (speedup 1.0000x reference)
"""Gromov-Wasserstein embedding loss kernel for 8x TRN2 NeuronCores.

Math (see reference):
  cos[i,j]  = (e1[i] . e2[j]) / (|e1[i]| |e2[j]| + eps)
  cost      = 1 - exp(cos - 1)
  d_w       = sum(cost * trans) = sum(trans) - sum(exp(cos-1) * trans)
  reg       = |E1^T E1 - I|_F^2 + |E2^T E2 - I|_F^2
  out       = [d_w, reg]

Sharding: rows of trans / cos split 8 ways (1024 rows per core).

Key trick: trans is folded into the exp via logs. Host ships
lnU = ln(trans * 2^26) in fp8; on device a scaled identity matmul
(I*256, bf16) preloads 256*lnU into PSUM, fp8 DoubleRow matmuls
accumulate 256*cos on top (host pre-normalizes embeddings, scales by
16, and pre-transposes into DoubleRow [128,2,N] layout), and a single
ACT pass computes exp(psum/256 - 1), yielding sum_j trans*exp(cos-1)
per row-block either via ACT's fused accumulator or via a DVE
row-reduce of the exp tile (the two engines are balanced ~60/40).
PE also accumulates the 256x256 grams of the x256-scaled fp8 row
shards (DoubleRow) for the regularizer, at the head of the schedule
where they double as the PE pstate warm-up. Host sums the tiny
partials.
"""

import sys

sys.path.insert(0, "/opt/trn_rl_repo")

import numpy as np

from concourse import bass, bacc, mybir
from concourse import tile
from concourse.bass_utils import run_bass_kernel_spmd

NCORES = 8
NUM = 8192
DIM = 256
SHARD = NUM // NCORES  # 1024 rows per core

BF16 = mybir.dt.bfloat16
F8 = mybir.dt.float8e4
F32 = mybir.dt.float32
NP_BF16 = mybir.dt.np(BF16)
NP_F8 = mybir.dt.np(F8)

LSCALE = 2.0**26  # trans prescale so ln(U) fits fp8 comfortably

_cached = {}


def build_program():
    nc = bacc.Bacc(None, target_bir_lowering=False)

    i2 = nc.declare_dram_parameter("i2", [128, 128], BF16, isOutput=False)
    nt = nc.declare_dram_parameter("nt", [128, 2, SHARD + NUM], F8, isOutput=False)
    lu = nc.declare_dram_parameter("lu", [SHARD, NUM], F8, isOutput=False)
    ems = nc.declare_dram_parameter("ems", [128, 2, 4, 2, DIM], F8, isOutput=False)
    outp = nc.declare_dram_parameter("out", [128, 1056], F32, isOutput=True)

    AF = mybir.ActivationFunctionType
    DR = mybir.MatmulPerfMode.DoubleRow

    with tile.TileContext(nc) as tc:
        with (
            tc.tile_pool(name="const", bufs=1) as constp,
            tc.tile_pool(name="stats", bufs=1) as statsp,
        ):
            cstt = constp.tile([128, 1], F32)
            nc.gpsimd.memset(cstt[:], -1.0)
            neg1 = cstt[:, 0:1]

            # tiny i2 first: it unblocks the PE pstate-ramp fillers; gram
            # shards right behind (their matmuls continue the warm-up)
            i2t = constp.tile([128, 128], BF16)
            nc.sync.dma_start(out=i2t[:], in_=i2[:, :])
            grpt = constp.tile([128, 2, 4, 2, DIM], F8)
            nc.sync.dma_start(out=grpt[:], in_=ems[:, :, :, :, :])
            ntt = constp.tile([128, 2, SHARD + NUM], F8)
            # minimal prefix: just n1 block 0 before the first lu tile; the
            # first emb2 chunk and the rest of n1 stream in behind it
            nc.sync.dma_start(out=ntt[:, :, 0:128], in_=nt[:, :, 0:128])
            n1tt = ntt[:, :, 0:SHARD]
            n2tt = ntt[:, :, SHARD : SHARD + NUM]

            # stage: gram quarters in [0:1024], d_w partials in [1024:1056]
            stage = statsp.tile([128, 1056], F32)
            accs = stage[:, 1024:1056]
            warm = statsp.tile([128, 1], F32)
            # dummy activation pulls the Exp table load off the critical path
            nc.scalar.activation(warm[:, 0:1], cstt[:, 0:1], AF.Exp, bias=neg1)

            with (
                tc.tile_pool(name="lut", bufs=3) as lup,
                tc.tile_pool(name="eout", bufs=4) as eop,
                tc.tile_pool(name="psumB", bufs=2, space="PSUM") as pbp,
            ):
                # ---- grams of raw shards (regularizer), at the head -------
                # fp8 DoubleRow (x256-scaled shards from host); each gram's
                # two quarters live in banks 0-1 of its own psum tile
                psg_a = pbp.tile([128, 2048], F32, tag="ps")
                psg_b = pbp.tile([128, 2048], F32, tag="ps")
                psg_list = [psg_a, psg_b]
                # tiny filler matmuls into free banks of psg1 start the PE
                # pstate ramp as soon as i2 arrives
                for w in range(24):
                    c0 = 1024 + (w % 4) * 64
                    nc.tensor.matmul(
                        psg_list[0][0:64, c0 : c0 + 64],
                        lhsT=i2t[:, 0:64],
                        rhs=i2t[:, 0:64],
                        start=True,
                        stop=True,
                        skip_group_check=True,
                    )
                for g in range(2):
                    psg = psg_list[g]
                    for h in range(2):
                        q = 2 * g + h
                        for b in range(4):
                            nc.tensor.matmul(
                                psg[:, h * 512 : h * 512 + DIM],
                                lhsT=grpt[:, g, b, :, h * 128 : (h + 1) * 128],
                                rhs=grpt[:, g, b, :, :],
                                start=(b == 0),
                                stop=(b == 3),
                                perf_mode=DR,
                                skip_group_check=True,
                            )
                        nc.vector.tensor_copy(
                            stage[:, q * DIM : (q + 1) * DIM],
                            psg[:, h * 512 : h * 512 + DIM],
                        )

                # ---------------- main loop: cos + exp + weighted reduce ---
                for jg in range(4):
                    for i in range(8):
                        lut = lup.tile([128, 2048], F8, tag="lu")
                        nc.sync.dma_start(
                            out=lut[:],
                            in_=lu[i * 128 : (i + 1) * 128, jg * 2048 : (jg + 1) * 2048],
                        )
                        if i == 0 and jg == 0:
                            nc.sync.dma_start(
                                out=ntt[:, :, SHARD : SHARD + 2048],
                                in_=nt[:, :, SHARD : SHARD + 2048],
                            )
                            nc.sync.dma_start(
                                out=ntt[:, :, 128:SHARD], in_=nt[:, :, 128:SHARD]
                            )
                        if i == 3 and jg == 0:
                            # gram results leave mid-stream, long after the
                            # drain copies complete
                            nc.sync.dma_start(out=outp[:, 0:512], in_=stage[:, 0:512])
                            nc.sync.dma_start(
                                out=outp[:, 512:1024], in_=stage[:, 512:1024]
                            )
                        if i == 4 and jg == 3:
                            # most accum columns are final by now; only the
                            # last four ride the tail
                            nc.sync.dma_start(
                                out=outp[:, 1024:1052], in_=stage[:, 1024:1052]
                            )
                        if i == 2 and jg < 3:
                            # next emb2-table chunk streams in behind this
                            # column group's third lnU tile
                            q = jg + 1
                            nc.sync.dma_start(
                                out=ntt[:, :, SHARD + q * 2048 : SHARD + (q + 1) * 2048],
                                in_=nt[:, :, SHARD + q * 2048 : SHARD + (q + 1) * 2048],
                            )
                        ps = pbp.tile([128, 2048], F32, tag="ps")
                        # preload 256*lnU into each 512-col psum bank
                        for jj in range(4):
                            nc.tensor.matmul(
                                ps[:, jj * 512 : (jj + 1) * 512],
                                lhsT=i2t[:],
                                rhs=lut[:, jj * 512 : (jj + 1) * 512],
                                start=True,
                                stop=False,
                                skip_group_check=True,
                            )
                        # accumulate 256*cos (fp8 DoubleRow, K=256 per instr)
                        for jj in range(4):
                            n0 = jg * 2048 + jj * 512
                            nc.tensor.matmul(
                                ps[:, jj * 512 : (jj + 1) * 512],
                                lhsT=n1tt[:, :, i * 128 : (i + 1) * 128],
                                rhs=n2tt[:, :, n0 : n0 + 512],
                                start=False,
                                stop=True,
                                perf_mode=DR,
                                skip_group_check=True,
                            )
                        # exp(psum/256 - 1) = trans*2^26 * exp(cos-1).
                        # Most tiles: ACT writes exp to SBUF and the (idle)
                        # DVE row-reduces it, skipping ACT's accumulator
                        # read; a few tiles (incl. the last four) keep the
                        # fused ACT accum to balance the engines and drain
                        # the DVE backlog before the tail.
                        t = jg * 8 + i
                        if t in (2, 7, 12, 17, 28, 29, 30, 31):
                            nc.scalar.activation(
                                ps[:],
                                ps[:],
                                AF.Exp,
                                bias=neg1,
                                scale=1.0 / 256.0,
                                accum_out=accs[:, t : t + 1],
                            )
                        else:
                            et = eop.tile([128, 2048], BF16, tag="et")
                            nc.scalar.activation(
                                et[:],
                                ps[:],
                                AF.Exp,
                                bias=neg1,
                                scale=1.0 / 256.0,
                            )
                            nc.vector.tensor_reduce(
                                out=accs[:, t : t + 1],
                                in_=et[:],
                                axis=mybir.AxisListType.X,
                                op=mybir.AluOpType.add,
                            )

                nc.sync.dma_start(out=outp[:, 1052:1056], in_=stage[:, 1052:1056])

    nc.finalize()
    return nc


def prepare(inputs):
    """Build (cached) program + per-core input maps. Returns (nc, in_maps, st)."""
    index1 = inputs["index1"]
    index2 = inputs["index2"]
    trans = inputs["trans"]
    emb1_w = inputs["emb1_w"]
    emb2_w = inputs["emb2_w"]
    # gather (identity for arange inputs, but stay correct in general)
    e1 = np.asarray(emb1_w, dtype=np.float32)[np.asarray(index1).astype(np.int64)]
    e2 = np.asarray(emb2_w, dtype=np.float32)[np.asarray(index2).astype(np.int64)]
    trans = np.ascontiguousarray(np.asarray(trans, dtype=np.float32))

    # sum(trans) on host (float64 accumulate)
    st = float(trans.sum(dtype=np.float64))

    # normalized, x16-scaled, fp8, transposed into DoubleRow [128, 2, N] layout
    def prep_table(e):
        n = e / (np.sqrt((e.astype(np.float64) ** 2).sum(1, keepdims=True)) + 1e-16)
        q = (n.astype(np.float32) * 16.0).astype(NP_F8)  # [N, 256]
        return np.ascontiguousarray(q.T.reshape(2, 128, -1).transpose(1, 0, 2))

    n1T = prep_table(e1)  # [128, 2, NUM]
    n2T = prep_table(e2)
    # per-core fused table: [n1 shard | full n2]
    ntall = [
        np.ascontiguousarray(
            np.concatenate([n1T[:, :, c * SHARD : (c + 1) * SHARD], n2T], axis=2)
        )
        for c in range(NCORES)
    ]

    # ln(trans * 2^26) in fp8 (clipped; exp() recovers trans*2^26)
    U = trans * np.float32(LSCALE)
    lnU = np.log(np.maximum(U, np.float32(1e-30)))
    np.maximum(lnU, np.float32(-50.0), out=lnU)
    lnU8 = lnU.astype(NP_F8)

    # x256-scaled fp8 shards in DoubleRow layout [128, 4, 2, 256] per core
    # (gram comes back x65536; host rescales)
    def prep_gram(e, c):
        q = (e[c * SHARD : (c + 1) * SHARD] * 256.0).astype(NP_F8)
        return np.ascontiguousarray(q.reshape(4, 2, 128, DIM).transpose(2, 0, 1, 3))

    emsb = [
        np.ascontiguousarray(
            np.stack([prep_gram(e1, c), prep_gram(e2, c)], axis=1)
        )
        for c in range(NCORES)
    ]

    if "nc" not in _cached:
        _cached["nc"] = build_program()
    nc = _cached["nc"]

    i2 = (np.eye(128, dtype=np.float32) * 256.0).astype(NP_BF16)
    in_maps = []
    for c in range(NCORES):
        in_maps.append(
            {
                "i2": i2,
                "nt": ntall[c],
                "lu": lnU8[c * SHARD : (c + 1) * SHARD],
                "ems": emsb[c],
            }
        )
    return nc, in_maps, st


def kernel(index1, index2, trans, emb1_w, emb2_w):
    nc, in_maps, st = prepare(
        dict(index1=index1, index2=index2, trans=trans, emb1_w=emb1_w, emb2_w=emb2_w)
    )

    res = run_bass_kernel_spmd(nc, in_maps, list(range(NCORES)))
    results = res.results

    syt = 0.0
    G1 = np.zeros((DIM, DIM), dtype=np.float64)
    G2 = np.zeros((DIM, DIM), dtype=np.float64)
    for c in range(NCORES):
        out = results[c]["out"].astype(np.float64)
        syt += float(out[:, 1024:1056].sum())
        G1 += np.concatenate([out[:, 0:256], out[:, 256:512]], axis=0)
        G2 += np.concatenate([out[:, 512:768], out[:, 768:1024]], axis=0)
    G1 /= 65536.0
    G2 /= 65536.0

    d_w = st - syt / LSCALE
    eye = np.eye(DIM, dtype=np.float64)
    reg = ((G1 - eye) ** 2).sum() + ((G2 - eye) ** 2).sum()
    return np.array([d_w, reg], dtype=np.float32)


# revision 37
# speedup vs baseline: 1.0005x; 1.0005x over previous
"""Gromov-Wasserstein embedding loss kernel for 8x TRN2 NeuronCores.

Math (see reference):
  cos[i,j]  = (e1[i] . e2[j]) / (|e1[i]| |e2[j]| + eps)
  cost      = 1 - exp(cos - 1)
  d_w       = sum(cost * trans) = sum(trans) - sum(exp(cos-1) * trans)
  reg       = |E1^T E1 - I|_F^2 + |E2^T E2 - I|_F^2
  out       = [d_w, reg]

Sharding: rows of trans / cos split 8 ways (1024 rows per core).

Key trick: trans is folded into the exp via logs. Host ships
lnU = ln(trans * 2^26) in fp8; on device a scaled identity matmul
(I*256, bf16) preloads 256*lnU into PSUM, fp8 DoubleRow matmuls
accumulate 256*cos on top (host pre-normalizes embeddings, scales by
16, and pre-transposes into DoubleRow [128,2,N] layout), and a single
ACT pass computes exp(psum/256 - 1), yielding sum_j trans*exp(cos-1)
per row-block either via ACT's fused accumulator or via a DVE
row-reduce of the exp tile (the two engines are balanced ~60/40).
PE also accumulates the 256x256 grams of the x256-scaled fp8 row
shards (DoubleRow) for the regularizer, at the head of the schedule
where they double as the PE pstate warm-up. Host sums the tiny
partials.
"""

import sys

sys.path.insert(0, "/opt/trn_rl_repo")

import numpy as np

from concourse import bass, bacc, mybir
from concourse import tile
from concourse.bass_utils import run_bass_kernel_spmd

NCORES = 8
NUM = 8192
DIM = 256
SHARD = NUM // NCORES  # 1024 rows per core

BF16 = mybir.dt.bfloat16
F8 = mybir.dt.float8e4
F32 = mybir.dt.float32
NP_BF16 = mybir.dt.np(BF16)
NP_F8 = mybir.dt.np(F8)

LSCALE = 2.0**26  # trans prescale so ln(U) fits fp8 comfortably

_cached = {}


def build_program():
    nc = bacc.Bacc(None, target_bir_lowering=False)

    i2 = nc.declare_dram_parameter("i2", [128, 128], BF16, isOutput=False)
    nt = nc.declare_dram_parameter("nt", [128, 2, SHARD + NUM], F8, isOutput=False)
    lu = nc.declare_dram_parameter("lu", [SHARD, NUM], F8, isOutput=False)
    ems = nc.declare_dram_parameter("ems", [128, 2, 4, 2, DIM], F8, isOutput=False)
    outp = nc.declare_dram_parameter("out", [128, 1056], F32, isOutput=True)

    AF = mybir.ActivationFunctionType
    DR = mybir.MatmulPerfMode.DoubleRow

    with tile.TileContext(nc) as tc:
        with (
            tc.tile_pool(name="const", bufs=1) as constp,
            tc.tile_pool(name="stats", bufs=1) as statsp,
        ):
            cstt = constp.tile([128, 1], F32)
            nc.gpsimd.memset(cstt[:], -1.0)
            neg1 = cstt[:, 0:1]

            # tiny i2 first: it unblocks the PE pstate-ramp fillers; gram
            # shards right behind (their matmuls continue the warm-up)
            i2t = constp.tile([128, 128], BF16)
            nc.sync.dma_start(out=i2t[:], in_=i2[:, :])
            grpt = constp.tile([128, 2, 4, 2, DIM], F8)
            nc.sync.dma_start(out=grpt[:], in_=ems[:, :, :, :, :])
            ntt = constp.tile([128, 2, SHARD + NUM], F8)
            # minimal prefix: just n1 block 0 before the first lu tile; the
            # first emb2 chunk and the rest of n1 stream in behind it
            nc.sync.dma_start(out=ntt[:, :, 0:128], in_=nt[:, :, 0:128])
            n1tt = ntt[:, :, 0:SHARD]
            n2tt = ntt[:, :, SHARD : SHARD + NUM]

            # stage: gram quarters in [0:1024], d_w partials in [1024:1056]
            stage = statsp.tile([128, 1056], F32)
            accs = stage[:, 1024:1056]
            warm = statsp.tile([128, 1], F32)
            # dummy activation pulls the Exp table load off the critical path
            nc.scalar.activation(warm[:, 0:1], cstt[:, 0:1], AF.Exp, bias=neg1)

            with (
                tc.tile_pool(name="lut", bufs=3) as lup,
                tc.tile_pool(name="eout", bufs=4) as eop,
                tc.tile_pool(name="psumB", bufs=2, space="PSUM") as pbp,
            ):
                # ---- grams of raw shards (regularizer), at the head -------
                # fp8 DoubleRow (x256-scaled shards from host); each gram's
                # two quarters live in banks 0-1 of its own psum tile
                psg = pbp.tile([128, 2048], F32, tag="ps")
                # a few early matmuls (weights only) start the PE pstate ramp
                # as soon as i2 arrives; all four gram quarters then fill the
                # four banks of ONE psum tile, so tile 0 gets the other
                # buffer with no wait, and all drain copies happen after the
                # last gram matmul (no WAR interleave)
                for w in range(24):
                    nc.tensor.matmul(
                        psg[0:64, 1024 + (w % 4) * 64 : 1088 + (w % 4) * 64],
                        lhsT=i2t[:, 0:64],
                        rhs=i2t[:, 0:64],
                        start=True,
                        stop=True,
                        skip_group_check=True,
                    )
                for g in range(2):
                    for h in range(2):
                        q = 2 * g + h
                        for b in range(4):
                            nc.tensor.matmul(
                                psg[:, q * 512 : q * 512 + DIM],
                                lhsT=grpt[:, g, b, :, h * 128 : (h + 1) * 128],
                                rhs=grpt[:, g, b, :, :],
                                start=(b == 0),
                                stop=(b == 3),
                                perf_mode=DR,
                                skip_group_check=True,
                            )
                for q in range(4):
                    nc.vector.tensor_copy(
                        stage[:, q * DIM : (q + 1) * DIM],
                        psg[:, q * 512 : q * 512 + DIM],
                    )

                # ---------------- main loop: cos + exp + weighted reduce ---
                for jg in range(4):
                    for i in range(8):
                        lut = lup.tile([128, 2048], F8, tag="lu")
                        nc.sync.dma_start(
                            out=lut[:],
                            in_=lu[i * 128 : (i + 1) * 128, jg * 2048 : (jg + 1) * 2048],
                        )
                        if i == 0 and jg == 0:
                            nc.sync.dma_start(
                                out=ntt[:, :, SHARD : SHARD + 2048],
                                in_=nt[:, :, SHARD : SHARD + 2048],
                            )
                            nc.sync.dma_start(
                                out=ntt[:, :, 128:SHARD], in_=nt[:, :, 128:SHARD]
                            )
                        if i == 3 and jg == 0:
                            # gram results leave mid-stream, long after the
                            # drain copies complete
                            nc.sync.dma_start(out=outp[:, 0:512], in_=stage[:, 0:512])
                            nc.sync.dma_start(
                                out=outp[:, 512:1024], in_=stage[:, 512:1024]
                            )
                        if i == 4 and jg == 3:
                            # most accum columns are final by now; only the
                            # last four ride the tail
                            nc.sync.dma_start(
                                out=outp[:, 1024:1052], in_=stage[:, 1024:1052]
                            )
                        if i == 2 and jg < 3:
                            # next emb2-table chunk streams in behind this
                            # column group's third lnU tile
                            q = jg + 1
                            nc.sync.dma_start(
                                out=ntt[:, :, SHARD + q * 2048 : SHARD + (q + 1) * 2048],
                                in_=nt[:, :, SHARD + q * 2048 : SHARD + (q + 1) * 2048],
                            )
                        ps = pbp.tile([128, 2048], F32, tag="ps")
                        # preload 256*lnU into each 512-col psum bank
                        for jj in range(4):
                            nc.tensor.matmul(
                                ps[:, jj * 512 : (jj + 1) * 512],
                                lhsT=i2t[:],
                                rhs=lut[:, jj * 512 : (jj + 1) * 512],
                                start=True,
                                stop=False,
                                skip_group_check=True,
                            )
                        # accumulate 256*cos (fp8 DoubleRow, K=256 per instr)
                        for jj in range(4):
                            n0 = jg * 2048 + jj * 512
                            nc.tensor.matmul(
                                ps[:, jj * 512 : (jj + 1) * 512],
                                lhsT=n1tt[:, :, i * 128 : (i + 1) * 128],
                                rhs=n2tt[:, :, n0 : n0 + 512],
                                start=False,
                                stop=True,
                                perf_mode=DR,
                                skip_group_check=True,
                            )
                        # exp(psum/256 - 1) = trans*2^26 * exp(cos-1).
                        # Most tiles: ACT writes exp to SBUF and the (idle)
                        # DVE row-reduces it, skipping ACT's accumulator
                        # read; a few tiles (incl. the last four) keep the
                        # fused ACT accum to balance the engines and drain
                        # the DVE backlog before the tail.
                        t = jg * 8 + i
                        if t in (2, 7, 12, 17, 28, 29, 30, 31):
                            nc.scalar.activation(
                                ps[:],
                                ps[:],
                                AF.Exp,
                                bias=neg1,
                                scale=1.0 / 256.0,
                                accum_out=accs[:, t : t + 1],
                            )
                        else:
                            et = eop.tile([128, 2048], BF16, tag="et")
                            nc.scalar.activation(
                                et[:],
                                ps[:],
                                AF.Exp,
                                bias=neg1,
                                scale=1.0 / 256.0,
                            )
                            nc.vector.tensor_reduce(
                                out=accs[:, t : t + 1],
                                in_=et[:],
                                axis=mybir.AxisListType.X,
                                op=mybir.AluOpType.add,
                            )

                nc.sync.dma_start(out=outp[:, 1052:1056], in_=stage[:, 1052:1056])

    nc.finalize()
    return nc


def prepare(inputs):
    """Build (cached) program + per-core input maps. Returns (nc, in_maps, st)."""
    index1 = inputs["index1"]
    index2 = inputs["index2"]
    trans = inputs["trans"]
    emb1_w = inputs["emb1_w"]
    emb2_w = inputs["emb2_w"]
    # gather (identity for arange inputs, but stay correct in general)
    e1 = np.asarray(emb1_w, dtype=np.float32)[np.asarray(index1).astype(np.int64)]
    e2 = np.asarray(emb2_w, dtype=np.float32)[np.asarray(index2).astype(np.int64)]
    trans = np.ascontiguousarray(np.asarray(trans, dtype=np.float32))

    # sum(trans) on host (float64 accumulate)
    st = float(trans.sum(dtype=np.float64))

    # normalized, x16-scaled, fp8, transposed into DoubleRow [128, 2, N] layout
    def prep_table(e):
        n = e / (np.sqrt((e.astype(np.float64) ** 2).sum(1, keepdims=True)) + 1e-16)
        q = (n.astype(np.float32) * 16.0).astype(NP_F8)  # [N, 256]
        return np.ascontiguousarray(q.T.reshape(2, 128, -1).transpose(1, 0, 2))

    n1T = prep_table(e1)  # [128, 2, NUM]
    n2T = prep_table(e2)
    # per-core fused table: [n1 shard | full n2]
    ntall = [
        np.ascontiguousarray(
            np.concatenate([n1T[:, :, c * SHARD : (c + 1) * SHARD], n2T], axis=2)
        )
        for c in range(NCORES)
    ]

    # ln(trans * 2^26) in fp8 (clipped; exp() recovers trans*2^26)
    U = trans * np.float32(LSCALE)
    lnU = np.log(np.maximum(U, np.float32(1e-30)))
    np.maximum(lnU, np.float32(-50.0), out=lnU)
    lnU8 = lnU.astype(NP_F8)

    # x256-scaled fp8 shards in DoubleRow layout [128, 4, 2, 256] per core
    # (gram comes back x65536; host rescales)
    def prep_gram(e, c):
        q = (e[c * SHARD : (c + 1) * SHARD] * 256.0).astype(NP_F8)
        return np.ascontiguousarray(q.reshape(4, 2, 128, DIM).transpose(2, 0, 1, 3))

    emsb = [
        np.ascontiguousarray(
            np.stack([prep_gram(e1, c), prep_gram(e2, c)], axis=1)
        )
        for c in range(NCORES)
    ]

    if "nc" not in _cached:
        _cached["nc"] = build_program()
    nc = _cached["nc"]

    i2 = (np.eye(128, dtype=np.float32) * 256.0).astype(NP_BF16)
    in_maps = []
    for c in range(NCORES):
        in_maps.append(
            {
                "i2": i2,
                "nt": ntall[c],
                "lu": lnU8[c * SHARD : (c + 1) * SHARD],
                "ems": emsb[c],
            }
        )
    return nc, in_maps, st


def kernel(index1, index2, trans, emb1_w, emb2_w):
    nc, in_maps, st = prepare(
        dict(index1=index1, index2=index2, trans=trans, emb1_w=emb1_w, emb2_w=emb2_w)
    )

    res = run_bass_kernel_spmd(nc, in_maps, list(range(NCORES)))
    results = res.results

    syt = 0.0
    G1 = np.zeros((DIM, DIM), dtype=np.float64)
    G2 = np.zeros((DIM, DIM), dtype=np.float64)
    for c in range(NCORES):
        out = results[c]["out"].astype(np.float64)
        syt += float(out[:, 1024:1056].sum())
        G1 += np.concatenate([out[:, 0:256], out[:, 256:512]], axis=0)
        G2 += np.concatenate([out[:, 512:768], out[:, 768:1024]], axis=0)
    G1 /= 65536.0
    G2 /= 65536.0

    d_w = st - syt / LSCALE
    eye = np.eye(DIM, dtype=np.float64)
    reg = ((G1 - eye) ** 2).sum() + ((G2 - eye) ** 2).sum()
    return np.array([d_w, reg], dtype=np.float32)


# revision 38
# speedup vs baseline: 1.0012x; 1.0007x over previous
"""Gromov-Wasserstein embedding loss kernel for 8x TRN2 NeuronCores.

Math (see reference):
  cos[i,j]  = (e1[i] . e2[j]) / (|e1[i]| |e2[j]| + eps)
  cost      = 1 - exp(cos - 1)
  d_w       = sum(cost * trans) = sum(trans) - sum(exp(cos-1) * trans)
  reg       = |E1^T E1 - I|_F^2 + |E2^T E2 - I|_F^2
  out       = [d_w, reg]

Sharding: rows of trans / cos split 8 ways (1024 rows per core).

Key trick: trans is folded into the exp via logs. Host ships
lnU = ln(trans * 2^26) in fp8; on device a scaled identity matmul
(I*256, bf16) preloads 256*lnU into PSUM, fp8 DoubleRow matmuls
accumulate 256*cos on top (host pre-normalizes embeddings, scales by
16, and pre-transposes into DoubleRow [128,2,N] layout), and a single
ACT pass computes exp(psum/256 - 1), yielding sum_j trans*exp(cos-1)
per row-block either via ACT's fused accumulator or via a DVE
row-reduce of the exp tile (the two engines are balanced ~60/40).
PE also accumulates the 256x256 grams of the x256-scaled fp8 row
shards (DoubleRow) for the regularizer, at the head of the schedule
where they double as the PE pstate warm-up. Host sums the tiny
partials.
"""

import sys

sys.path.insert(0, "/opt/trn_rl_repo")

import numpy as np

from concourse import bass, bacc, mybir
from concourse import tile
from concourse.bass_utils import run_bass_kernel_spmd

NCORES = 8
NUM = 8192
DIM = 256
SHARD = NUM // NCORES  # 1024 rows per core

BF16 = mybir.dt.bfloat16
F8 = mybir.dt.float8e4
F32 = mybir.dt.float32
NP_BF16 = mybir.dt.np(BF16)
NP_F8 = mybir.dt.np(F8)

LSCALE = 2.0**26  # trans prescale so ln(U) fits fp8 comfortably

_cached = {}


def build_program():
    nc = bacc.Bacc(None, target_bir_lowering=False)

    i2 = nc.declare_dram_parameter("i2", [128, 128], BF16, isOutput=False)
    nt = nc.declare_dram_parameter("nt", [128, 2, SHARD + NUM], F8, isOutput=False)
    lu = nc.declare_dram_parameter("lu", [SHARD, NUM], F8, isOutput=False)
    ems = nc.declare_dram_parameter("ems", [128, 2, 4, 2, DIM], F8, isOutput=False)
    outp = nc.declare_dram_parameter("out", [128, 1056], F32, isOutput=True)

    AF = mybir.ActivationFunctionType
    DR = mybir.MatmulPerfMode.DoubleRow

    with tile.TileContext(nc) as tc:
        with (
            tc.tile_pool(name="const", bufs=1) as constp,
            tc.tile_pool(name="stats", bufs=1) as statsp,
        ):
            cstt = constp.tile([128, 1], F32)
            nc.gpsimd.memset(cstt[:], -1.0)
            neg1 = cstt[:, 0:1]

            # tiny i2 first: it unblocks the PE pstate-ramp fillers; gram
            # shards right behind (their matmuls continue the warm-up)
            i2t = constp.tile([128, 128], BF16)
            nc.sync.dma_start(out=i2t[:], in_=i2[:, :])
            grpt = constp.tile([128, 2, 4, 2, DIM], F8)
            nc.sync.dma_start(out=grpt[:], in_=ems[:, :, :, :, :])
            ntt = constp.tile([128, 2, SHARD + NUM], F8)
            # minimal prefix: just n1 block 0 before the first lu tile; the
            # first emb2 chunk and the rest of n1 stream in behind it
            nc.sync.dma_start(out=ntt[:, :, 0:128], in_=nt[:, :, 0:128])
            n1tt = ntt[:, :, 0:SHARD]
            n2tt = ntt[:, :, SHARD : SHARD + NUM]

            # stage: gram quarters in [0:1024], d_w partials in [1024:1056]
            stage = statsp.tile([128, 1056], F32)
            accs = stage[:, 1024:1056]
            warm = statsp.tile([128, 1], F32)
            # dummy activation pulls the Exp table load off the critical path
            nc.scalar.activation(warm[:, 0:1], cstt[:, 0:1], AF.Exp, bias=neg1)

            with (
                tc.tile_pool(name="lut", bufs=3) as lup,
                tc.tile_pool(name="eout", bufs=4) as eop,
                tc.tile_pool(name="psumB", bufs=2, space="PSUM") as pbp,
            ):
                # ---- grams of raw shards (regularizer), at the head -------
                # fp8 DoubleRow (x256-scaled shards from host); each gram's
                # two quarters live in banks 0-1 of its own psum tile
                psg = pbp.tile([128, 2048], F32, tag="ps")
                # a few early matmuls (weights only) start the PE pstate ramp
                # as soon as i2 arrives; all four gram quarters then fill the
                # four banks of ONE psum tile, so tile 0 gets the other
                # buffer with no wait, and all drain copies happen after the
                # last gram matmul (no WAR interleave)
                for w in range(24):
                    nc.tensor.matmul(
                        psg[0:64, 1024 + (w % 4) * 64 : 1088 + (w % 4) * 64],
                        lhsT=i2t[:, 0:64],
                        rhs=i2t[:, 0:64],
                        start=True,
                        stop=True,
                        skip_group_check=True,
                    )
                for g in range(2):
                    for h in range(2):
                        q = 2 * g + h
                        for b in range(4):
                            nc.tensor.matmul(
                                psg[:, q * 512 : q * 512 + DIM],
                                lhsT=grpt[:, g, b, :, h * 128 : (h + 1) * 128],
                                rhs=grpt[:, g, b, :, :],
                                start=(b == 0),
                                stop=(b == 3),
                                perf_mode=DR,
                                skip_group_check=True,
                            )
                for q in range(4):
                    nc.vector.tensor_copy(
                        stage[:, q * DIM : (q + 1) * DIM],
                        psg[:, q * 512 : q * 512 + DIM],
                    )

                # ---------------- main loop: cos + exp + weighted reduce ---
                for jg in range(4):
                    for i in range(8):
                        lut = lup.tile([128, 2048], F8, tag="lu")
                        nc.sync.dma_start(
                            out=lut[:],
                            in_=lu[i * 128 : (i + 1) * 128, jg * 2048 : (jg + 1) * 2048],
                        )
                        if i == 0 and jg == 0:
                            # first chunk in two halves: the first two DR
                            # matmuls overlap the second half's transfer
                            nc.sync.dma_start(
                                out=ntt[:, :, SHARD : SHARD + 1024],
                                in_=nt[:, :, SHARD : SHARD + 1024],
                            )
                            nc.sync.dma_start(
                                out=ntt[:, :, SHARD + 1024 : SHARD + 2048],
                                in_=nt[:, :, SHARD + 1024 : SHARD + 2048],
                            )
                            nc.sync.dma_start(
                                out=ntt[:, :, 128:SHARD], in_=nt[:, :, 128:SHARD]
                            )
                        if i == 3 and jg == 0:
                            # gram results leave mid-stream, long after the
                            # drain copies complete
                            nc.sync.dma_start(out=outp[:, 0:512], in_=stage[:, 0:512])
                            nc.sync.dma_start(
                                out=outp[:, 512:1024], in_=stage[:, 512:1024]
                            )
                        if i == 4 and jg == 3:
                            # most accum columns are final by now; only the
                            # last four ride the tail
                            nc.sync.dma_start(
                                out=outp[:, 1024:1052], in_=stage[:, 1024:1052]
                            )
                        if i == 2 and jg < 3:
                            # next emb2-table chunk streams in behind this
                            # column group's third lnU tile
                            q = jg + 1
                            nc.sync.dma_start(
                                out=ntt[:, :, SHARD + q * 2048 : SHARD + (q + 1) * 2048],
                                in_=nt[:, :, SHARD + q * 2048 : SHARD + (q + 1) * 2048],
                            )
                        ps = pbp.tile([128, 2048], F32, tag="ps")
                        # preload 256*lnU into each 512-col psum bank
                        for jj in range(4):
                            nc.tensor.matmul(
                                ps[:, jj * 512 : (jj + 1) * 512],
                                lhsT=i2t[:],
                                rhs=lut[:, jj * 512 : (jj + 1) * 512],
                                start=True,
                                stop=False,
                                skip_group_check=True,
                            )
                        # accumulate 256*cos (fp8 DoubleRow, K=256 per instr)
                        for jj in range(4):
                            n0 = jg * 2048 + jj * 512
                            nc.tensor.matmul(
                                ps[:, jj * 512 : (jj + 1) * 512],
                                lhsT=n1tt[:, :, i * 128 : (i + 1) * 128],
                                rhs=n2tt[:, :, n0 : n0 + 512],
                                start=False,
                                stop=True,
                                perf_mode=DR,
                                skip_group_check=True,
                            )
                        # exp(psum/256 - 1) = trans*2^26 * exp(cos-1).
                        # Most tiles: ACT writes exp to SBUF and the (idle)
                        # DVE row-reduces it, skipping ACT's accumulator
                        # read; a few tiles (incl. the last four) keep the
                        # fused ACT accum to balance the engines and drain
                        # the DVE backlog before the tail.
                        t = jg * 8 + i
                        if t in (2, 7, 12, 17, 28, 29, 30, 31):
                            nc.scalar.activation(
                                ps[:],
                                ps[:],
                                AF.Exp,
                                bias=neg1,
                                scale=1.0 / 256.0,
                                accum_out=accs[:, t : t + 1],
                            )
                        else:
                            et = eop.tile([128, 2048], BF16, tag="et")
                            nc.scalar.activation(
                                et[:],
                                ps[:],
                                AF.Exp,
                                bias=neg1,
                                scale=1.0 / 256.0,
                            )
                            nc.vector.tensor_reduce(
                                out=accs[:, t : t + 1],
                                in_=et[:],
                                axis=mybir.AxisListType.X,
                                op=mybir.AluOpType.add,
                            )

                nc.sync.dma_start(out=outp[:, 1052:1056], in_=stage[:, 1052:1056])

    nc.finalize()
    return nc


def prepare(inputs):
    """Build (cached) program + per-core input maps. Returns (nc, in_maps, st)."""
    index1 = inputs["index1"]
    index2 = inputs["index2"]
    trans = inputs["trans"]
    emb1_w = inputs["emb1_w"]
    emb2_w = inputs["emb2_w"]
    # gather (identity for arange inputs, but stay correct in general)
    e1 = np.asarray(emb1_w, dtype=np.float32)[np.asarray(index1).astype(np.int64)]
    e2 = np.asarray(emb2_w, dtype=np.float32)[np.asarray(index2).astype(np.int64)]
    trans = np.ascontiguousarray(np.asarray(trans, dtype=np.float32))

    # sum(trans) on host (float64 accumulate)
    st = float(trans.sum(dtype=np.float64))

    # normalized, x16-scaled, fp8, transposed into DoubleRow [128, 2, N] layout
    def prep_table(e):
        n = e / (np.sqrt((e.astype(np.float64) ** 2).sum(1, keepdims=True)) + 1e-16)
        q = (n.astype(np.float32) * 16.0).astype(NP_F8)  # [N, 256]
        return np.ascontiguousarray(q.T.reshape(2, 128, -1).transpose(1, 0, 2))

    n1T = prep_table(e1)  # [128, 2, NUM]
    n2T = prep_table(e2)
    # per-core fused table: [n1 shard | full n2]
    ntall = [
        np.ascontiguousarray(
            np.concatenate([n1T[:, :, c * SHARD : (c + 1) * SHARD], n2T], axis=2)
        )
        for c in range(NCORES)
    ]

    # ln(trans * 2^26) in fp8 (clipped; exp() recovers trans*2^26)
    U = trans * np.float32(LSCALE)
    lnU = np.log(np.maximum(U, np.float32(1e-30)))
    np.maximum(lnU, np.float32(-50.0), out=lnU)
    lnU8 = lnU.astype(NP_F8)

    # x256-scaled fp8 shards in DoubleRow layout [128, 4, 2, 256] per core
    # (gram comes back x65536; host rescales)
    def prep_gram(e, c):
        q = (e[c * SHARD : (c + 1) * SHARD] * 256.0).astype(NP_F8)
        return np.ascontiguousarray(q.reshape(4, 2, 128, DIM).transpose(2, 0, 1, 3))

    emsb = [
        np.ascontiguousarray(
            np.stack([prep_gram(e1, c), prep_gram(e2, c)], axis=1)
        )
        for c in range(NCORES)
    ]

    if "nc" not in _cached:
        _cached["nc"] = build_program()
    nc = _cached["nc"]

    i2 = (np.eye(128, dtype=np.float32) * 256.0).astype(NP_BF16)
    in_maps = []
    for c in range(NCORES):
        in_maps.append(
            {
                "i2": i2,
                "nt": ntall[c],
                "lu": lnU8[c * SHARD : (c + 1) * SHARD],
                "ems": emsb[c],
            }
        )
    return nc, in_maps, st


def kernel(index1, index2, trans, emb1_w, emb2_w):
    nc, in_maps, st = prepare(
        dict(index1=index1, index2=index2, trans=trans, emb1_w=emb1_w, emb2_w=emb2_w)
    )

    res = run_bass_kernel_spmd(nc, in_maps, list(range(NCORES)))
    results = res.results

    syt = 0.0
    G1 = np.zeros((DIM, DIM), dtype=np.float64)
    G2 = np.zeros((DIM, DIM), dtype=np.float64)
    for c in range(NCORES):
        out = results[c]["out"].astype(np.float64)
        syt += float(out[:, 1024:1056].sum())
        G1 += np.concatenate([out[:, 0:256], out[:, 256:512]], axis=0)
        G2 += np.concatenate([out[:, 512:768], out[:, 768:1024]], axis=0)
    G1 /= 65536.0
    G2 /= 65536.0

    d_w = st - syt / LSCALE
    eye = np.eye(DIM, dtype=np.float64)
    reg = ((G1 - eye) ** 2).sum() + ((G2 - eye) ** 2).sum()
    return np.array([d_w, reg], dtype=np.float32)
